# revision 8
# baseline (speedup 1.0000x reference)
"""ComplexSympNet Trainium2 kernel.

Math: the reference layer is, in complex form (z_q = q_r + i q_i, etc.):
    mix   = alpha * z_q + beta * z_p                (alpha = a_r + i a_i, beta = b_r + i b_i)
    t     = tanh_split(W @ mix + c)                 (W = Wr + i Wi, tanh applied per re/im part)
    z2    = DW @ t + i*bias                         (DW = diag * W)
    z_q  += beta * z2 ; z_p -= alpha * z2
    zc_q += z_q      ; zc_p += z_p

Everything linear is folded (on the host) into per-layer real matrices:
  - forward: psum_re/psum_im accumulate 4 matmuls each over the 4 state
    tiles (q_r, q_i, p_r, p_i) with combined weights.
  - backward: each state's delta is 2 matmuls (over tanh re/im outputs).
  - rank-1 bias terms become per-partition bias vectors folded into the
    ScalarE activation that materializes states from PSUM.

Layout: feature-major [128 features, batch] so the contraction dim is on
partitions; the host transposes inputs/outputs. Batch is sharded over the
8 cores (data parallel); per-layer weights are replicated.

On-device state lives in persistent PSUM banks: the Tensor engine first
writes q0 via an identity matmul (plain fp32, start=True), then each
layer's backward matmuls accumulate the delta in place.  ScalarE/VectorE
materialize the state to SBUF (adding the cumulative bias) for the next
layer's forward matmuls, and VectorE accumulates the running output sum.
Matmuls use float32r (full-rate fp32 PE mode); the only precision-critical
path (q0 -> 9*q0) stays exact fp32.
"""

import os

import numpy as np

import concourse.bass as bass
import concourse.bacc as bacc
import concourse.mybir as mybir
from concourse.bass import ts
from concourse.bass_utils import run_bass_kernel_spmd
from concourse.tile import TileContext

B, N, L, NL = 65536, 128, 128, 8
NCORES = 8
BC = B // NCORES          # batch columns per core
F = 256                   # batch columns per tile (half a PSUM bank)
NT = BC // F              # tiles per core (processed as pairs of chains)

f32 = mybir.dt.float32
f32r = mybir.dt.float32r
Tanh = mybir.ActivationFunctionType.Tanh
Ident = mybir.ActivationFunctionType.Identity

_PROGRAM = None           # (nc, dram names)
LAST_RESULTS = None       # BassKernelResults of the most recent run


def _build_program():
    nc = bacc.Bacc("TRN2", target_bir_lowering=False)
    S = nc.declare_dram_parameter("S", [2, 128, 2 * BC], f32r, isOutput=False)
    WF = nc.declare_dram_parameter("WF", [128, NL * 8 * 128], f32r, isOutput=False)
    WB = nc.declare_dram_parameter("WB", [128, NL * 8 * 128], f32r, isOutput=False)
    TB = nc.declare_dram_parameter("TB", [128, 2 * NL], f32, isOutput=False)
    CB = nc.declare_dram_parameter("CB", [128, 4 * NL], f32, isOutput=False)
    EYE = nc.declare_dram_parameter("EYE", [128, 128], f32, isOutput=False)
    OUT = nc.declare_dram_parameter("OUT", [2, 128, 2 * BC], f32, isOutput=True)

    def wf_blk(l, j, s):
        blk = (l * 2 + j) * 4 + s
        return slice(blk * 128, (blk + 1) * 128)

    def wb_blk(l, s, t):
        blk = (l * 4 + s) * 2 + t
        return slice(blk * 128, (blk + 1) * 128)

    with TileContext(nc) as tc:
        with (
            tc.tile_pool(name="wp", bufs=1) as wp,
            tc.tile_pool(name="io", bufs=4) as io,
            tc.tile_pool(name="wk", bufs=3) as wk,
            tc.tile_pool(name="fps", bufs=2, space="PSUM") as fps,
            tc.tile_pool(name="sps", bufs=1, space="PSUM") as sps,
        ):
            wf = wp.tile([128, NL * 8 * 128], f32r, name="wf")
            nc.sync.dma_start(wf, WF[:, :])
            wb = wp.tile([128, NL * 8 * 128], f32r, name="wb")
            nc.sync.dma_start(wb, WB[:, :])
            tb = wp.tile([128, 2 * NL], f32, name="tb")
            nc.sync.dma_start(tb, TB[:, :])
            cb = wp.tile([128, 4 * NL], f32, name="cb")
            nc.sync.dma_start(cb, CB[:, :])
            eye = wp.tile([128, 128], f32, name="eye")
            nc.sync.dma_start(eye, EYE[:, :])

            # Engine instructions can carry only ONE semaphore wait each;
            # absorb every weight-DMA completion on a throwaway op of the
            # consuming engine so no compute instruction ever needs two.
            warm = fps.tile([128, 2 * F], f32, tag="ps12_0", name="warm")
            nc.tensor.matmul(warm[:, 0:2], wf[:, 0:128], wf[:, 0:2], start=True, stop=False)
            nc.tensor.matmul(warm[:, 0:2], wb[:, 0:128], wb[:, 0:2], start=False, stop=False)
            nc.tensor.matmul(warm[:, 0:2], eye[:, :], eye[:, 0:2].bitcast(f32), start=False, stop=True)
            scr = wp.tile([128, 1], f32, name="scr")
            nc.scalar.copy(scr, tb[:, 0:1])
            nc.scalar.copy(scr, cb[:, 0:1])
            dscr = wp.tile([128, 1], f32, name="dscr")

            for k in range(NT // 2):
                ch = []
                for c in range(2):
                    it = 2 * k + c
                    qin = [
                        io.tile([128, 2 * F], f32r, tag=f"qin{p}_{c}", name=f"qin{p}_{c}_{it}")
                        for p in range(2)
                    ]
                    qa = [
                        io.tile([128, 2 * F], f32, tag=f"qa{p}_{c}", name=f"qa{p}_{c}_{it}")
                        for p in range(2)
                    ]
                    for p in range(2):
                        nc.sync.dma_start(qin[p], S[p, :, ts(it, 2 * F)])
                        # accumulator starts as the raw input; gpsimd DMA casts
                        # the f32r-typed DRAM view to a plain f32 tile (same bytes)
                        nc.gpsimd.dma_start(qa[p], S[p, :, ts(it, 2 * F)])
                        # absorb the qa-DMA wait on DVE so the accumulate
                        # tensor_tensor only waits on ScalarE
                        nc.vector.tensor_copy(dscr, qa[p][:, 0:1])
                    spsum = [
                        sps.tile([128, 2 * F], f32, tag=f"sps{p}_{c}", name=f"sps{p}_{c}_{it}")
                        for p in range(2)
                    ]
                    ch.append(dict(it=it, qin=qin, qa=qa, spsum=spsum, cur=None))

                for l in range(NL):
                    # --- forward matmuls (and state-psum init on layer 0) ---
                    for c in range(2):
                        ps12 = fps.tile(
                            [128, 2 * F], f32, tag=f"ps12_{c}", name=f"ps12_{c}_{k}_{l}"
                        )
                        cur = ch[c]["cur"]
                        if cur is None:
                            qin = ch[c]["qin"]
                            cur = [
                                qin[0][:, 0:F], qin[0][:, F : 2 * F],
                                qin[1][:, 0:F], qin[1][:, F : 2 * F],
                            ]
                        for s in range(4):
                            for j in range(2):
                                nc.tensor.matmul(
                                    ps12[:, j * F : (j + 1) * F],
                                    wf[:, wf_blk(l, j, s)],
                                    cur[s],
                                    start=(s == 0 and j == 0),
                                    stop=(s == 3 and j == 1),
                                )
                        if l == 0:
                            for p in range(2):
                                nc.tensor.matmul(
                                    ch[c]["spsum"][p],
                                    eye[:, :],
                                    ch[c]["qin"][p].bitcast(f32),
                                    start=True,
                                    stop=False,
                                )
                        ch[c]["ps12"] = ps12

                    # --- tanh ---
                    for c in range(2):
                        ps12 = ch[c]["ps12"]
                        r_ = wk.tile([128, F], f32r, tag=f"r_{c}", name=f"r_{c}_{k}_{l}")
                        i_ = wk.tile([128, F], f32r, tag=f"i_{c}", name=f"i_{c}_{k}_{l}")
                        nc.scalar.activation(
                            r_, ps12[:, 0:F], Tanh,
                            bias=tb[:, 2 * l : 2 * l + 1], scale=1.0,
                        )
                        nc.scalar.activation(
                            i_, ps12[:, F : 2 * F], Tanh,
                            bias=tb[:, 2 * l + 1 : 2 * l + 2], scale=1.0,
                        )
                        ch[c]["ri"] = (r_, i_)

                    # --- backward matmuls: accumulate deltas onto state psums ---
                    for c in range(2):
                        r_, i_ = ch[c]["ri"]
                        for s in range(4):
                            p, h = divmod(s, 2)
                            out_ap = ch[c]["spsum"][p][:, h * F : (h + 1) * F]
                            last_bank_mm = l == NL - 1 and h == 1
                            nc.tensor.matmul(
                                out_ap,
                                wb[:, wb_blk(l, s, 0)],
                                r_,
                                start=False,
                                stop=False,
                            )
                            nc.tensor.matmul(
                                out_ap,
                                wb[:, wb_blk(l, s, 1)],
                                i_,
                                start=False,
                                stop=last_bank_mm,
                            )

                    # --- materialize states to SBUF (+cumulative bias), accumulate ---
                    for c in range(2):
                        st01 = wk.tile([128, 2 * F], f32r, tag=f"st01_{c}", name=f"st01_{c}_{k}_{l}")
                        st23 = wk.tile([128, 2 * F], f32r, tag=f"st23_{c}", name=f"st23_{c}_{k}_{l}")
                        sts = [
                            st01[:, 0:F], st01[:, F : 2 * F],
                            st23[:, 0:F], st23[:, F : 2 * F],
                        ]
                        for s in range(4):
                            p, h = divmod(s, 2)
                            src = ch[c]["spsum"][p][:, h * F : (h + 1) * F]
                            bias_ap = cb[:, s * NL + l : s * NL + l + 1]
                            nc.scalar.activation(sts[s], src, Ident, bias=bias_ap, scale=1.0)
                        nc.vector.tensor_add(ch[c]["qa"][0], ch[c]["qa"][0], st01.bitcast(f32))
                        nc.vector.tensor_add(ch[c]["qa"][1], ch[c]["qa"][1], st23.bitcast(f32))
                        ch[c]["cur"] = sts

                for c in range(2):
                    it = ch[c]["it"]
                    for p in range(2):
                        nc.sync.dma_start(OUT[p, :, ts(it, 2 * F)], ch[c]["qa"][p])

    nc.compile()
    return nc


def _derive_host_tensors(inputs):
    """Fold all per-layer scalars/biases into matmul weights (float64)."""
    a = np.asarray(inputs["a"], np.float64)
    Wr = np.asarray(inputs["Wr"], np.float64)
    Wi = np.asarray(inputs["Wi"], np.float64)
    br = np.asarray(inputs["br"], np.float64)
    bi = np.asarray(inputs["bi"], np.float64)
    bias = np.asarray(inputs["bias"], np.float64)
    diag = np.asarray(inputs["diag"], np.float64)

    WFm = np.zeros((NL, 2, 4, 128, 128))   # [l, psum_j, state_s, L, N]
    WBm = np.zeros((NL, 4, 2, 128, 128))   # [l, state_s, (r_,i_), L, N]
    TB = np.zeros((128, 2 * NL))
    CBstep = np.zeros((4, NL, 128))

    for l in range(NL):
        ar, ai, br_s, bi_s = a[l]
        W_r, W_i = Wr[l], Wi[l]
        DWr = diag[l] * W_r
        DWi = diag[l] * W_i

        # forward: psum0 = arg of tanh -> real_, psum1 -> imag_
        WFm[l, 0, 0] = ar * W_r - ai * W_i
        WFm[l, 0, 1] = -(ai * W_r + ar * W_i)
        WFm[l, 0, 2] = br_s * W_r - bi_s * W_i
        WFm[l, 0, 3] = -(bi_s * W_r + br_s * W_i)
        WFm[l, 1, 0] = ai * W_r + ar * W_i
        WFm[l, 1, 1] = ar * W_r - ai * W_i
        WFm[l, 1, 2] = bi_s * W_r + br_s * W_i
        WFm[l, 1, 3] = br_s * W_r - bi_s * W_i

        # backward deltas per state (s: 0=q_r, 1=q_i, 2=p_r, 3=p_i)
        WBm[l, 0, 0] = br_s * DWr - bi_s * DWi
        WBm[l, 0, 1] = -(br_s * DWi + bi_s * DWr)
        WBm[l, 1, 0] = br_s * DWi + bi_s * DWr
        WBm[l, 1, 1] = br_s * DWr - bi_s * DWi
        WBm[l, 2, 0] = -ar * DWr + ai * DWi
        WBm[l, 2, 1] = ar * DWi + ai * DWr
        WBm[l, 3, 0] = -(ar * DWi + ai * DWr)
        WBm[l, 3, 1] = -ar * DWr + ai * DWi

        TB[:, 2 * l] = br[l] - bi[l]
        TB[:, 2 * l + 1] = br[l] + bi[l]

        CBstep[0, l] = -bi_s * bias[l]
        CBstep[1, l] = br_s * bias[l]
        CBstep[2, l] = ai * bias[l]
        CBstep[3, l] = -ar * bias[l]

    CBcum = np.cumsum(CBstep, axis=1)            # [4, NL, 128]
    CB = CBcum.transpose(2, 0, 1).reshape(128, 4 * NL)

    # lhsT layouts: forward needs the transpose ([N, L]); backward is natural.
    WF_flat = np.ascontiguousarray(
        WFm.transpose(4, 0, 1, 2, 3).reshape(128, NL * 8 * 128), np.float32
    )
    WB_flat = np.ascontiguousarray(
        WBm.transpose(3, 0, 1, 2, 4).reshape(128, NL * 8 * 128), np.float32
    )
    return dict(
        WF=WF_flat,
        WB=WB_flat,
        TB=np.ascontiguousarray(TB, np.float32),
        CB=np.ascontiguousarray(CB, np.float32),
        EYE=np.eye(128, dtype=np.float32),
    )


def _pack_states(inputs):
    """[B,N] inputs -> per-core pair-packed feature-major [2, 128, 2*BC]."""
    Ts = [np.asarray(inputs[k], np.float32).T for k in ("q_r", "q_i", "p_r", "p_i")]
    per_core = []
    for c in range(NCORES):
        sl = slice(c * BC, (c + 1) * BC)
        S = np.empty((2, 128, 2 * BC), np.float32)
        v = S.reshape(2, 128, NT, 2, F)
        for p in range(2):
            for h in range(2):
                v[p, :, :, h, :] = Ts[2 * p + h][:, sl].reshape(128, NT, F)
        per_core.append(S)
    return per_core


def _unpack_out(results):
    """Per-core OUT [2,128,2*BC] -> full [4, B, N] in reference order."""
    accs = [np.empty((128, B), np.float32) for _ in range(4)]  # s-order qr,qi,pr,pi
    for c, res in enumerate(results):
        o = np.asarray(res["OUT"]).reshape(2, 128, NT, 2, F)
        sl = slice(c * BC, (c + 1) * BC)
        for p in range(2):
            for h in range(2):
                accs[2 * p + h][:, sl] = o[p, :, :, h, :].reshape(128, BC)
    # reference stacks [pc_r, pc_i, qc_r, qc_i]
    return np.stack([accs[2].T, accs[3].T, accs[0].T, accs[1].T])


def kernel(**inputs) -> np.ndarray:
    global _PROGRAM, LAST_RESULTS
    if _PROGRAM is None:
        _PROGRAM = _build_program()
    nc = _PROGRAM

    host = _derive_host_tensors(inputs)
    states = _pack_states(inputs)
    in_maps = [{**host, "S": states[c]} for c in range(NCORES)]

    trace = os.environ.get("BASS_KERNEL_TRACE", "0") == "1"
    res = run_bass_kernel_spmd(nc, in_maps, list(range(NCORES)), trace=trace)
    LAST_RESULTS = res
    return _unpack_out(res.results)


# revision 15
# speedup vs baseline: 151.9108x; 151.9108x over previous
"""ComplexSympNet Trainium2 kernel.

Math: the reference layer is, in complex form (z_q = q_r + i q_i, etc.):
    mix   = alpha * z_q + beta * z_p                (alpha = a_r + i a_i, beta = b_r + i b_i)
    t     = tanh_split(W @ mix + c)                 (W = Wr + i Wi, tanh applied per re/im part)
    z2    = DW @ t + i*bias                         (DW = diag * W)
    z_q  += beta * z2 ; z_p -= alpha * z2
    zc_q += z_q      ; zc_p += z_p

Everything linear is folded (on the host) into per-layer real matrices:
  - forward: psum_re/psum_im accumulate 4 matmuls each over the 4 state
    tiles (q_r, q_i, p_r, p_i) with combined weights.
  - backward: each state's delta is 2 matmuls (over tanh re/im outputs).
  - rank-1 bias terms become per-partition bias vectors folded into the
    ScalarE activation that materializes states from PSUM.

Layout: feature-major [128 features, batch] so the contraction dim is on
partitions; the host transposes inputs/outputs. Batch is sharded over the
8 cores (data parallel); per-layer weights are replicated.

On-device state lives in persistent PSUM banks: the Tensor engine first
writes q0 via an identity matmul (plain fp32, start=True), then each
layer's backward matmuls accumulate the delta in place.  ScalarE/VectorE
materialize the state to SBUF (adding the cumulative bias) for the next
layer's forward matmuls, and VectorE accumulates the running output sum.
Matmuls use float32r (full-rate fp32 PE mode); the only precision-critical
path (q0 -> 9*q0) stays exact fp32.
"""

import os

import numpy as np

import concourse.bass as bass
import concourse.bacc as bacc
import concourse.mybir as mybir
from concourse.bass import ts
from concourse.bass_utils import run_bass_kernel_spmd
from concourse.tile import TileContext

B, N, L, NL = 65536, 128, 128, 8
NCORES = 8
BC = B // NCORES          # batch columns per core
F = 256                   # batch columns per tile (half a PSUM bank)
NT = BC // F              # tiles per core (processed as pairs of chains)

f32 = mybir.dt.float32
f32r = mybir.dt.float32r
Tanh = mybir.ActivationFunctionType.Tanh
Ident = mybir.ActivationFunctionType.Identity

LAST_RESULTS = None       # BassKernelResults of the most recent run


def _build_program(zero_bias=False, no_dscr=False, wk_bufs=3, io_bufs=4, dve_copy=False, acc_pool=0, ipass_f32r=False, last_direct=False, j_outer=False):
    nc = bacc.Bacc("TRN2", target_bir_lowering=False)
    S = nc.declare_dram_parameter("S", [2, 128, 2 * BC], f32r, isOutput=False)
    WF = nc.declare_dram_parameter("WF", [128, NL * 8 * 128], f32r, isOutput=False)
    WB = nc.declare_dram_parameter("WB", [128, NL * 8 * 128], f32r, isOutput=False)
    TB = nc.declare_dram_parameter("TB", [128, 2 * NL], f32, isOutput=False)
    CB = nc.declare_dram_parameter("CB", [128, 4 * NL], f32, isOutput=False)
    EYE = nc.declare_dram_parameter("EYE", [128, 128], f32r, isOutput=False)
    OUT = nc.declare_dram_parameter("OUT", [2, 128, 2 * BC], f32, isOutput=True)

    def wf_blk(l, j, s):
        blk = (l * 2 + j) * 4 + s
        return slice(blk * 128, (blk + 1) * 128)

    def wb_blk(l, s, t):
        blk = (l * 4 + s) * 2 + t
        return slice(blk * 128, (blk + 1) * 128)

    with TileContext(nc) as tc:
        with (
            tc.tile_pool(name="wp", bufs=1) as wp,
            tc.tile_pool(name="io", bufs=io_bufs) as io,
            tc.tile_pool(name="wk", bufs=wk_bufs) as wk,
            tc.tile_pool(name="fps", bufs=2, space="PSUM") as fps,
            tc.tile_pool(name="sps", bufs=1, space="PSUM") as sps,
        ):
            wf = wp.tile([128, NL * 8 * 128], f32r, name="wf")
            nc.sync.dma_start(wf, WF[:, :])
            wb = wp.tile([128, NL * 8 * 128], f32r, name="wb")
            nc.sync.dma_start(wb, WB[:, :])
            tb = wp.tile([128, 2 * NL], f32, name="tb")
            nc.sync.dma_start(tb, TB[:, :])
            cb = wp.tile([128, 4 * NL], f32, name="cb")
            nc.sync.dma_start(cb, CB[:, :])
            eye = wp.tile([128, 128], f32r, name="eye")
            nc.sync.dma_start(eye, EYE[:, :])

            # Engine instructions can carry only ONE semaphore wait each;
            # absorb every weight-DMA completion on a throwaway op of the
            # consuming engine so no compute instruction ever needs two.
            warm = fps.tile([128, 2 * F], f32, tag="ps12_0", name="warm")
            nc.tensor.matmul(warm[:, 0:2], wf[:, 0:128], wf[:, 0:2], start=True, stop=False)
            nc.tensor.matmul(warm[:, 0:2], wb[:, 0:128], wb[:, 0:2], start=False, stop=False)
            nc.tensor.matmul(warm[:, 0:2], eye[:, :], eye[:, 0:2], start=False, stop=True)
            scr = wp.tile([128, 1], f32, name="scr")
            nc.scalar.copy(scr, tb[:, 0:1])
            nc.scalar.copy(scr, cb[:, 0:1])
            dscr = wp.tile([128, 1], f32, name="dscr")

            for k in range(NT // 2):
                ch = []
                for c in range(2):
                    it = 2 * k + c
                    qin = [
                        io.tile([128, 2 * F], f32r, tag=f"qin{p}_{c}", name=f"qin{p}_{c}_{it}")
                        for p in range(2)
                    ]
                    qa = [
                        io.tile([128, 2 * F], f32, tag=f"qa{p}_{c}", name=f"qa{p}_{c}_{it}", bufs=3)
                        for p in range(2)
                    ]
                    for p in range(2):
                        nc.sync.dma_start(qin[p], S[p, :, ts(it, 2 * F)])
                        # accumulator starts as the raw input; gpsimd DMA casts
                        # the f32r-typed DRAM view to a plain f32 tile (same bytes)
                        nc.gpsimd.dma_start(qa[p], S[p, :, ts(it, 2 * F)])
                        if not no_dscr:
                            # absorb the qa-DMA wait on DVE so the accumulate
                            # tensor_tensor only waits on ScalarE
                            nc.vector.tensor_copy(dscr, qa[p][:, 0:1])
                    spsum = [
                        sps.tile([128, 2 * F], f32, tag=f"sps{p}_{c}", name=f"sps{p}_{c}_{it}")
                        for p in range(2)
                    ]
                    ch.append(dict(it=it, qin=qin, qa=qa, spsum=spsum, cur=None))

                for l in range(NL):
                    # --- forward matmuls (and state-psum init on layer 0) ---
                    for c in range(2):
                        ps12 = fps.tile(
                            [128, 2 * F], f32, tag=f"ps12_{c}", name=f"ps12_{c}_{k}_{l}"
                        )
                        cur = ch[c]["cur"]
                        if cur is None:
                            qin = ch[c]["qin"]
                            cur = [
                                qin[0][:, 0:F], qin[0][:, F : 2 * F],
                                qin[1][:, 0:F], qin[1][:, F : 2 * F],
                            ]
                        order = (
                            [(s, j) for j in range(2) for s in range(4)]
                            if j_outer
                            else [(s, j) for s in range(4) for j in range(2)]
                        )
                        for n_i, (s, j) in enumerate(order):
                            nc.tensor.matmul(
                                ps12[:, j * F : (j + 1) * F],
                                wf[:, wf_blk(l, j, s)],
                                cur[s],
                                start=(n_i == 0),
                                stop=(n_i == 7),
                            )
                        if l == 0:
                            for p in range(2):
                                if ipass_f32r:
                                    nc.tensor.matmul(
                                        ch[c]["spsum"][p],
                                        eye[:, :],
                                        ch[c]["qin"][p],
                                        start=True,
                                        stop=False,
                                    )
                                else:
                                    nc.tensor.matmul(
                                        ch[c]["spsum"][p],
                                        eye[:, :].bitcast(f32),
                                        ch[c]["qin"][p].bitcast(f32),
                                        start=True,
                                        stop=False,
                                    )
                        ch[c]["ps12"] = ps12

                    # --- tanh ---
                    for c in range(2):
                        ps12 = ch[c]["ps12"]
                        r_ = wk.tile([128, F], f32r, tag=f"r_{c}", name=f"r_{c}_{k}_{l}")
                        i_ = wk.tile([128, F], f32r, tag=f"i_{c}", name=f"i_{c}_{k}_{l}")
                        nc.scalar.activation(
                            r_, ps12[:, 0:F], Tanh,
                            bias=tb[:, 2 * l : 2 * l + 1], scale=1.0,
                        )
                        nc.scalar.activation(
                            i_, ps12[:, F : 2 * F], Tanh,
                            bias=tb[:, 2 * l + 1 : 2 * l + 2], scale=1.0,
                        )
                        ch[c]["ri"] = (r_, i_)

                    # --- backward matmuls: accumulate deltas onto state psums ---
                    for c in range(2):
                        r_, i_ = ch[c]["ri"]
                        for s in range(4):
                            p, h = divmod(s, 2)
                            out_ap = ch[c]["spsum"][p][:, h * F : (h + 1) * F]
                            last_bank_mm = l == NL - 1 and h == 1
                            nc.tensor.matmul(
                                out_ap,
                                wb[:, wb_blk(l, s, 0)],
                                r_,
                                start=False,
                                stop=False,
                            )
                            nc.tensor.matmul(
                                out_ap,
                                wb[:, wb_blk(l, s, 1)],
                                i_,
                                start=False,
                                stop=last_bank_mm,
                            )

                    # --- materialize states to SBUF (+cumulative bias), accumulate ---
                    if last_direct and l == NL - 1:
                        for c in range(2):
                            for p in range(2):
                                if c < acc_pool:
                                    nc.gpsimd.tensor_tensor(
                                        ch[c]["qa"][p], ch[c]["qa"][p],
                                        ch[c]["spsum"][p], mybir.AluOpType.add,
                                    )
                                else:
                                    nc.vector.tensor_add(
                                        ch[c]["qa"][p], ch[c]["qa"][p], ch[c]["spsum"][p]
                                    )
                        continue
                    for c in range(2):
                        st01 = wk.tile([128, 2 * F], f32r, tag=f"st01_{c}", name=f"st01_{c}_{k}_{l}")
                        st23 = wk.tile([128, 2 * F], f32r, tag=f"st23_{c}", name=f"st23_{c}_{k}_{l}")
                        sts = [
                            st01[:, 0:F], st01[:, F : 2 * F],
                            st23[:, 0:F], st23[:, F : 2 * F],
                        ]
                        if zero_bias:
                            # br/bias are all-zero: one full-bank copy per
                            # state pair, no per-partition bias needed
                            if dve_copy:
                                nc.scalar.copy(st01, ch[c]["spsum"][0])
                                nc.vector.tensor_copy(st23, ch[c]["spsum"][1])
                            else:
                                nc.scalar.copy(st01, ch[c]["spsum"][0])
                                nc.scalar.copy(st23, ch[c]["spsum"][1])
                        else:
                            for s in range(4):
                                p, h = divmod(s, 2)
                                src = ch[c]["spsum"][p][:, h * F : (h + 1) * F]
                                bias_ap = cb[:, s * NL + l : s * NL + l + 1]
                                nc.scalar.activation(sts[s], src, Ident, bias=bias_ap, scale=1.0)
                        if c < acc_pool:
                            nc.gpsimd.tensor_tensor(
                                ch[c]["qa"][0], ch[c]["qa"][0], st01.bitcast(f32),
                                mybir.AluOpType.add,
                            )
                            nc.gpsimd.tensor_tensor(
                                ch[c]["qa"][1], ch[c]["qa"][1], st23.bitcast(f32),
                                mybir.AluOpType.add,
                            )
                        else:
                            nc.vector.tensor_add(ch[c]["qa"][0], ch[c]["qa"][0], st01.bitcast(f32))
                            nc.vector.tensor_add(ch[c]["qa"][1], ch[c]["qa"][1], st23.bitcast(f32))
                        ch[c]["cur"] = sts

                for c in range(2):
                    it = ch[c]["it"]
                    for p in range(2):
                        nc.sync.dma_start(OUT[p, :, ts(it, 2 * F)], ch[c]["qa"][p])

    nc.compile()
    return nc


def _derive_host_tensors(inputs):
    """Fold all per-layer scalars/biases into matmul weights (float64)."""
    a = np.asarray(inputs["a"], np.float64)
    Wr = np.asarray(inputs["Wr"], np.float64)
    Wi = np.asarray(inputs["Wi"], np.float64)
    br = np.asarray(inputs["br"], np.float64)
    bi = np.asarray(inputs["bi"], np.float64)
    bias = np.asarray(inputs["bias"], np.float64)
    diag = np.asarray(inputs["diag"], np.float64)

    WFm = np.zeros((NL, 2, 4, 128, 128))   # [l, psum_j, state_s, L, N]
    WBm = np.zeros((NL, 4, 2, 128, 128))   # [l, state_s, (r_,i_), L, N]
    TB = np.zeros((128, 2 * NL))
    CBstep = np.zeros((4, NL, 128))

    for l in range(NL):
        ar, ai, br_s, bi_s = a[l]
        W_r, W_i = Wr[l], Wi[l]
        DWr = diag[l] * W_r
        DWi = diag[l] * W_i

        # forward: psum0 = arg of tanh -> real_, psum1 -> imag_
        WFm[l, 0, 0] = ar * W_r - ai * W_i
        WFm[l, 0, 1] = -(ai * W_r + ar * W_i)
        WFm[l, 0, 2] = br_s * W_r - bi_s * W_i
        WFm[l, 0, 3] = -(bi_s * W_r + br_s * W_i)
        WFm[l, 1, 0] = ai * W_r + ar * W_i
        WFm[l, 1, 1] = ar * W_r - ai * W_i
        WFm[l, 1, 2] = bi_s * W_r + br_s * W_i
        WFm[l, 1, 3] = br_s * W_r - bi_s * W_i

        # backward deltas per state (s: 0=q_r, 1=q_i, 2=p_r, 3=p_i)
        WBm[l, 0, 0] = br_s * DWr - bi_s * DWi
        WBm[l, 0, 1] = -(br_s * DWi + bi_s * DWr)
        WBm[l, 1, 0] = br_s * DWi + bi_s * DWr
        WBm[l, 1, 1] = br_s * DWr - bi_s * DWi
        WBm[l, 2, 0] = -ar * DWr + ai * DWi
        WBm[l, 2, 1] = ar * DWi + ai * DWr
        WBm[l, 3, 0] = -(ar * DWi + ai * DWr)
        WBm[l, 3, 1] = -ar * DWr + ai * DWi

        TB[:, 2 * l] = br[l] - bi[l]
        TB[:, 2 * l + 1] = br[l] + bi[l]

        CBstep[0, l] = -bi_s * bias[l]
        CBstep[1, l] = br_s * bias[l]
        CBstep[2, l] = ai * bias[l]
        CBstep[3, l] = -ar * bias[l]

    CBcum = np.cumsum(CBstep, axis=1)            # [4, NL, 128]
    CB = CBcum.transpose(2, 0, 1).reshape(128, 4 * NL)

    # lhsT layouts: forward needs the transpose ([N, L]); backward is natural.
    WF_flat = np.ascontiguousarray(
        WFm.transpose(4, 0, 1, 2, 3).reshape(128, NL * 8 * 128), np.float32
    )
    WB_flat = np.ascontiguousarray(
        WBm.transpose(3, 0, 1, 2, 4).reshape(128, NL * 8 * 128), np.float32
    )
    return dict(
        WF=WF_flat,
        WB=WB_flat,
        TB=np.ascontiguousarray(TB, np.float32),
        CB=np.ascontiguousarray(CB, np.float32),
        EYE=np.eye(128, dtype=np.float32),
    )


def _pack_states(inputs):
    """[B,N] inputs -> per-core pair-packed feature-major [2, 128, 2*BC]."""
    Ts = [np.asarray(inputs[k], np.float32).T for k in ("q_r", "q_i", "p_r", "p_i")]
    per_core = []
    for c in range(NCORES):
        sl = slice(c * BC, (c + 1) * BC)
        S = np.empty((2, 128, 2 * BC), np.float32)
        v = S.reshape(2, 128, NT, 2, F)
        for p in range(2):
            for h in range(2):
                v[p, :, :, h, :] = Ts[2 * p + h][:, sl].reshape(128, NT, F)
        per_core.append(S)
    return per_core


def _unpack_out(results):
    """Per-core OUT [2,128,2*BC] -> full [4, B, N] in reference order."""
    accs = [np.empty((128, B), np.float32) for _ in range(4)]  # s-order qr,qi,pr,pi
    for c, res in enumerate(results):
        o = np.asarray(res["OUT"]).reshape(2, 128, NT, 2, F)
        sl = slice(c * BC, (c + 1) * BC)
        for p in range(2):
            for h in range(2):
                accs[2 * p + h][:, sl] = o[p, :, :, h, :].reshape(128, BC)
    # reference stacks [pc_r, pc_i, qc_r, qc_i]
    return np.stack([accs[2].T, accs[3].T, accs[0].T, accs[1].T])


_PROGRAMS = {}


def kernel(**inputs) -> np.ndarray:
    global LAST_RESULTS

    host = _derive_host_tensors(inputs)
    # fast path when the rank-1 bias terms vanish (br and bias are zeros in
    # this problem); general path otherwise
    fast = bool(np.all(host["CB"] == 0.0))
    key = ("fast" if fast else "general")
    if key not in _PROGRAMS:
        if fast:
            _PROGRAMS[key] = _build_program(
                no_dscr=True, zero_bias=True, dve_copy=True,
                acc_pool=1, ipass_f32r=True,
            )
        else:
            _PROGRAMS[key] = _build_program()
    nc = _PROGRAMS[key]
    states = _pack_states(inputs)
    in_maps = [{**host, "S": states[c]} for c in range(NCORES)]

    trace = os.environ.get("BASS_KERNEL_TRACE", "0") == "1"
    res = run_bass_kernel_spmd(nc, in_maps, list(range(NCORES)), trace=trace)
    LAST_RESULTS = res
    return _unpack_out(res.results)
